# revision 1
# baseline (speedup 1.0000x reference)
"""Trainium2 Bass kernel for nn_Block_68633577390412.

Pipeline (reference): channel mix (64x64) -> frame into 256 half-overlapping
512-windows -> rfft -> per-(c,k) IIR over frames y_f = (s_f + y_{f-1})*t ->
irfft -> hann window -> overlap-add -> tanh(gain*.)

Sharding: 8 cores = 4 batches x 2 channel-halves, no cross-core traffic.

Per-core design:
  - rfft/irfft are dense packed-real DFT matmuls (512 real lanes per frame:
    Re k=0..256, Im k=1..255) on the tensor engine in f32r, which streams at
    1 cycle/row for moving dims >= 256 (measured ~104 ns per 128x128x256
    matmul, weight self-load fully overlapped).
  - The channel mix runs as K=128 bf16 matmuls using an exact hi/lo split of
    x and the mixer stacked along the contraction dim (error ~2^-16),
    accumulating in fp32 PSUM; the mixed signal x'' lives in SBUF with time
    on partitions so the framed windowed^T views are plain strided SBUF APs:
    framing costs no DMA and no transpose.
  - The frame recurrence is the DVE TensorTensorScan instruction
    (state = (s + state) * t) with a stride-0 broadcast multiplier, reading
    the DFT result straight from PSUM.
  - irfft, hann window, overlap-add and the one-frame OLA shift are fused
    into two accumulating matmuls per output block against precomputed
    E1/E2 = (irfft-basis * hann * gain) halves; the shift is an off-by-one
    column view of the scan output.
  - tanh runs on the scalar engine directly from PSUM; output DMA is fully
    contiguous.
  - The spectral phase is split into two frame-halves (scan state carried
    across halves via a small SBUF tile) and interleaved with the mix: the
    tensor engine starts DFT work once the first ~8.5MB of the signal is
    mixed, and the remaining mix groups are spread between spectral pairs
    to fill scan-wait gaps; the second frame-half is software-pipelined by
    one pair so each pair's irfft overlaps the next pair's scans. Input DMA
    streams through both DGE families (SWDGE + ACT-engine HWDGE) in
    parallel.
  - TRN2 instructions carry a single sync-wait slot and this walrus build
    rejects multi-wait instructions, so a post-pass hoists excess Tile-
    assigned waits onto standalone event-semaphore waits on the same engine.
"""
import numpy as np
import ml_dtypes
from contextlib import ExitStack

import concourse.bass as bass
import concourse.tile as tile
from concourse import mybir
from concourse.bass_utils import run_bass_kernel_spmd

F32 = mybir.dt.float32
F32R = mybir.dt.float32r
BF16 = mybir.dt.bfloat16

B, C, T = 4, 64, 65536
WS, STEP, K, NF = 512, 256, 257, 256
NCORES = 8
CH = 32            # channels per core
NCHUNK = 512       # 128-sample time chunks in T
XG = 32            # x tile groups (2048 samples each)


def _build_matrices():
    w = np.arange(WS, dtype=np.float64)
    r = np.arange(512, dtype=np.float64)
    # D[w, r]: packed rfft basis. r<257: Re(spec[r]); r>=257: Im(spec[r-256]).
    D = np.empty((WS, 512), dtype=np.float64)
    D[:, :257] = np.cos(2 * np.pi * np.outer(w, r[:257]) / WS)
    D[:, 257:] = -np.sin(2 * np.pi * np.outer(w, r[257:] - 256) / WS)
    # G[r, w]: packed irfft basis.
    G = np.empty((512, WS), dtype=np.float64)
    G[0, :] = 1.0 / WS
    kk = r[1:256]
    G[1:256, :] = 2.0 * np.cos(2 * np.pi * np.outer(kk, w) / WS) / WS
    G[256, :] = np.cos(np.pi * w) / WS
    G[257:, :] = -2.0 * np.sin(2 * np.pi * np.outer(kk, w) / WS) / WS
    hann = 0.5 * (1.0 - np.cos(2.0 * np.pi * w / WS))
    E1 = G[:, :STEP] * hann[None, :STEP]   # first half-window
    E2 = G[:, STEP:] * hann[None, STEP:]   # second half-window
    return D, E1, E2


def _split_excess_waits(nc):
    """TRN2 instructions have a single sync-wait slot. Tile's semaphore
    assignment can attach several waits to one instruction; hoist the extras
    onto standalone event-semaphore wait instructions on the same engine,
    placed immediately before the instruction (same engine stream => same
    semantics)."""
    ctr = 0
    for fn in nc.m.functions:
        for bb in fn.blocks:
            out = []
            changed = False
            for inst in bb.instructions:
                si = inst.sync_info
                ow = list(si.on_wait) if si and si.on_wait else []
                if len(ow) > 1:
                    for w in ow[:-1]:
                        ev = mybir.InstEventSemaphore(
                            name=f"WSPLIT-{ctr}", ins=[], outs=[])
                        ctr += 1
                        ev.engine = inst.engine
                        evsi = ev.sync_info
                        if evsi is None:
                            ev.sync_info = mybir.SyncInfo(on_wait=[w], on_update=[])
                        else:
                            evsi.on_wait = [w]
                        out.append(ev)
                    si.on_wait = [ow[-1]]
                    changed = True
                out.append(inst)
            if changed:
                bb.instructions = out
    return ctr


def _build_program(split=True, loop_n=1):
    nc = bass.Bass("TRN2", target_bir_lowering=False, debug=False,
                   num_devices=NCORES)
    xpk_t = nc.dram_tensor("xpk", [2 * C, T], BF16, kind="ExternalInput")
    msb_t = nc.dram_tensor("msb", [128, 2 * CH], BF16, kind="ExternalInput")
    dmat_t = nc.dram_tensor("dmat", [512, 512], F32R, kind="ExternalInput")
    emat_t = nc.dram_tensor("emat", [512, 512], F32R, kind="ExternalInput")
    tpk_t = nc.dram_tensor("tpk", [512, CH], F32, kind="ExternalInput")
    out_t = nc.dram_tensor("out", [CH, 256, 256], F32, kind="ExternalOutput")

    xpk = xpk_t.ap()
    msb, dmat, emat, tpk = msb_t.ap(), dmat_t.ap(), emat_t.ap(), tpk_t.ap()
    out = out_t.ap()

    with tile.TileContext(nc) as tc, ExitStack() as ctx:
        cpool = ctx.enter_context(tc.tile_pool(name="const", bufs=1))
        m_sb = cpool.tile([128, 2 * CH], BF16, tag="m_sb")
        nc.sync.dma_start(m_sb[:], msb[:])
        dm_sb = cpool.tile([128, 2048], F32R, tag="dm_sb")
        em_sb = cpool.tile([128, 2048], F32R, tag="em_sb")
        tp_sb = cpool.tile([128, 4 * CH], F32, tag="tp_sb")
        for i in range(4):
            nc.sync.dma_start(dm_sb[:, 512 * i:512 * (i + 1)],
                              dmat[128 * i:128 * (i + 1), :])
            nc.sync.dma_start(em_sb[:, 512 * i:512 * (i + 1)],
                              emat[128 * i:128 * (i + 1), :])
            nc.sync.dma_start(tp_sb[:, CH * i:CH * (i + 1)],
                              tpk[128 * i:128 * (i + 1), :])

        # x'' : mixed signal, time-on-partitions. col = 32*chunk + d_local.
        xsq = cpool.tile([128, 32 * (NCHUNK + 2)], F32R, tag="xsq")
        nc.gpsimd.memset(xsq[:, 32 * NCHUNK:].bitcast(F32), 0.0)

        # ---- Interleaved mix + spectral, split by frame halves ----
        # Spectral work for frames [0,128) only needs the first ~8.5MB of the
        # mixed signal, so it starts as soon as mix groups 0..16 are done and
        # overlaps the remaining input DMA + mix work.
        xpool = ctx.enter_context(tc.tile_pool(name="xin", bufs=8))
        mixpool = ctx.enter_context(tc.tile_pool(name="mixps", bufs=2,
                                                 space="PSUM"))
        spool = ctx.enter_context(tc.tile_pool(name="sps", bufs=4, space="PSUM"))
        opool = ctx.enter_context(tc.tile_pool(name="ops", bufs=2, space="PSUM"))
        ypool = ctx.enter_context(tc.tile_pool(name="y", bufs=16))
        outpool = ctx.enter_context(tc.tile_pool(name="outs", bufs=8))
        # per-(pair,channel,r-chunk) scan carry between frame halves
        carry = cpool.tile([128, 8 * CH], F32, tag="carry")

        # DVE absorber: make the vector engine observe the tp_sb load queues
        # before the scans.
        tp_scratch = cpool.tile([128, CH], F32, tag="tp_scratch")
        for i in range(4):
            nc.vector.tensor_copy(tp_scratch[:], tp_sb[:, CH * i:CH * (i + 1)])

        def mix_group(g):
            xt = xpool.tile([128, 2048], BF16, tag="xt")
            # split each fill across SWDGE and the ACT-engine HWDGE path so
            # the two DGE families stream the input in parallel
            nc.gpsimd.dma_start(xt[0:64, :], xpk[0:64, 2048 * g:2048 * (g + 1)])
            nc.scalar.dma_start(xt[64:128, :], xpk[64:128, 2048 * g:2048 * (g + 1)])
            mps = mixpool.tile([128, 512], F32, tag="mps")
            for m in range(16):
                lhsT = xt[:, 128 * m:128 * (m + 1)]
                sl = mps[:, 32 * m:32 * (m + 1)]
                nc.tensor.matmul(sl, lhsT, m_sb[:, 0:CH], start=True, stop=False)
                nc.tensor.matmul(sl, lhsT, m_sb[:, CH:2 * CH],
                                 start=False, stop=True)
            nc.scalar.copy(xsq[:, 512 * g:512 * (g + 1)], mps[:])

        def spectral_front(p, half):
            fbase = 64 * 128 * half
            # DFT: s[r, f-half] for the channel pair; one matmul per
            # (r-chunk, w-chunk) with a 2D moving AP spanning both channels
            # (N=256 keeps the f32r fast path).
            s_tiles = []
            for ri in range(4):
                sps = spool.tile([128, 256], F32, tag="sps")
                for wi in range(4):
                    lhsT = dm_sb[:, 512 * wi + 128 * ri:512 * wi + 128 * (ri + 1)]
                    v = xsq[:, 32 * wi + 2 * p + fbase:
                            32 * wi + 2 * p + fbase + 2]
                    rhs = bass.AP(v.tensor, v.offset,
                                  [list(q) for q in v.ap] + [[64, 128]])
                    nc.tensor.matmul(sps[:], lhsT, rhs,
                                     start=(wi == 0), stop=(wi == 3))
                s_tiles.append(sps)

            # scan along frames within the half; carry chains the halves
            y_tiles = {}
            for c01 in range(2):
                d_local = 2 * p + c01
                for ri in range(4):
                    yt = ypool.tile([128, 129], F32R, tag="y")
                    cidx = p * 8 + c01 * 4 + ri
                    t_col = tp_sb[:, CH * ri + d_local:CH * ri + d_local + 1]
                    if half == 0:
                        nc.vector.memset(yt[:, 0:1].bitcast(F32), 0.0)
                        nc.vector.tensor_tensor_scan(
                            yt[:, 1:129],
                            s_tiles[ri][:, 128 * c01:128 * (c01 + 1)],
                            t_col.broadcast_to((128, 128)),
                            0.0, mybir.AluOpType.add, mybir.AluOpType.mult)
                        nc.vector.tensor_copy(carry[:, cidx:cidx + 1],
                                              yt[:, 128:129].bitcast(F32))
                    else:
                        nc.vector.tensor_copy(yt[:, 0:1],
                                              carry[:, cidx:cidx + 1].bitcast(F32R))
                        nc.vector.tensor_tensor_scan(
                            yt[:, 1:129],
                            s_tiles[ri][:, 128 * c01:128 * (c01 + 1)],
                            t_col.broadcast_to((128, 128)),
                            carry[:, cidx:cidx + 1],
                            mybir.AluOpType.add, mybir.AluOpType.mult)
                    y_tiles[(c01, ri)] = yt
            return y_tiles

        def spectral_back(p, half, y_tiles):
            # iDFT + hann + OLA for this half's q-chunk
            ops = opool.tile([128, 512], F32, tag="ops")
            for c01 in range(2):
                sl = ops[:, 256 * c01:256 * (c01 + 1)]
                for ri in range(4):
                    yt = y_tiles[(c01, ri)]
                    nc.tensor.matmul(sl, yt[:, 1:129],
                                     em_sb[:, 512 * ri:512 * ri + 256],
                                     start=(ri == 0), stop=False)
                    nc.tensor.matmul(sl, yt[:, 0:128],
                                     em_sb[:, 512 * ri + 256:512 * (ri + 1)],
                                     start=False, stop=(ri == 3))
            for c01 in range(2):
                ot = outpool.tile([128, 256], F32, tag="ot")
                nc.scalar.activation(ot[:], ops[:, 256 * c01:256 * (c01 + 1)],
                                     mybir.ActivationFunctionType.Tanh)
                nc.gpsimd.dma_start(
                    out[2 * p + c01, 128 * half:128 * (half + 1), :], ot[:])

        GSPLIT = 17          # mix groups needed before frame-half 0
        for g in range(GSPLIT):
            mix_group(g)
        # interleave the remaining mix groups between spectral pairs so the
        # tensor engine has independent work whenever a pair briefly waits
        # on its scans
        g_next = GSPLIT
        for p in range(CH // 2):
            y0 = spectral_front(p, 0)
            spectral_back(p, 0, y0)
            if g_next < XG:
                mix_group(g_next)
                g_next += 1
        while g_next < XG:
            mix_group(g_next)
            g_next += 1
        # half 1 has no independent filler work, so pipeline it by one pair:
        # the irfft of pair p-1 runs while pair p's scans complete.
        pend = None
        for p in range(CH // 2):
            y1 = spectral_front(p, 1)
            if pend is not None:
                spectral_back(pend[0], 1, pend[1])
            pend = (p, y1)
        spectral_back(pend[0], 1, pend[1])
    if split:
        _split_excess_waits(nc)
    return nc


_CACHE = {}


def _get_program():
    if "nc" not in _CACHE:
        _CACHE["nc"] = _build_program()
    return _CACHE["nc"]


def _host_inputs(x, mixer, transfer, gain):
    D, E1, E2 = _build_matrices()
    g = float(np.asarray(gain).reshape(-1)[0])
    dmat = np.ascontiguousarray(D, dtype=np.float32)
    emat = np.concatenate([E1 * g, E2 * g], axis=1).astype(np.float32)
    emat = np.ascontiguousarray(emat)

    x = np.asarray(x, dtype=np.float32)
    mixer = np.asarray(mixer, dtype=np.float32)
    transfer = np.asarray(transfer, dtype=np.float32)

    xhi = x.astype(ml_dtypes.bfloat16)
    xlo = (x - xhi.astype(np.float32)).astype(ml_dtypes.bfloat16)
    mhi = mixer.astype(ml_dtypes.bfloat16)
    mlo = (mixer - mhi.astype(np.float32)).astype(ml_dtypes.bfloat16)

    in_maps = []
    for core in range(NCORES):
        b, h = divmod(core, 2)
        msb = np.zeros((128, 2 * CH), dtype=ml_dtypes.bfloat16)
        msb[0:64, 0:CH] = mhi[:, CH * h:CH * (h + 1)]
        msb[64:128, 0:CH] = mhi[:, CH * h:CH * (h + 1)]
        msb[0:64, CH:2 * CH] = mlo[:, CH * h:CH * (h + 1)]
        msb[64:128, CH:2 * CH] = mlo[:, CH * h:CH * (h + 1)]
        tr = transfer[CH * h:CH * (h + 1)]          # [32, 257]
        tpk = np.zeros((512, CH), dtype=np.float32)
        tpk[:257, :] = tr.T
        tpk[257:, :] = tr[:, 1:256].T
        in_maps.append({
            "xpk": np.ascontiguousarray(np.concatenate([xhi[b], xlo[b]], axis=0)),
            "msb": msb,
            "dmat": dmat,
            "emat": emat,
            "tpk": tpk,
        })
    return in_maps


def _run(in_maps, trace=False):
    nc = _get_program()
    return run_bass_kernel_spmd(nc, in_maps, list(range(NCORES)), trace=trace)


def _cached_exec(nc):
    """jax.jit-cached variant of bass2jax.run_bass_via_pjrt (which rebuilds
    its jit closure every call)."""
    if getattr(nc, "_cached_exec", None) is not None:
        return nc._cached_exec
    import jax
    import numpy as _np
    from jax.sharding import Mesh, PartitionSpec
    from jax.experimental.shard_map import shard_map
    from concourse import bass2jax as b2j
    from concourse import mybir as _mybir
    b2j.install_neuronx_cc_hook()
    partition_name = nc.partition_id_tensor.name if nc.partition_id_tensor else None
    in_names, out_names, out_avals, zero_shapes = [], [], [], []
    for alloc in nc.m.functions[0].allocations:
        if not isinstance(alloc, _mybir.MemoryLocationSet):
            continue
        name = alloc.memorylocations[0].name
        if alloc.kind == "ExternalInput":
            if name != partition_name:
                in_names.append(name)
        elif alloc.kind == "ExternalOutput":
            out_names.append(name)
            shape = tuple(alloc.tensor_shape)
            dtype = _mybir.dt.np(alloc.dtype)
            out_avals.append(jax.core.ShapedArray(shape, dtype))
            zero_shapes.append((shape, dtype))
    n_params = len(in_names)
    all_in = list(in_names) + list(out_names)
    if partition_name is not None:
        all_in.append(partition_name)

    def _body(*args):
        operands = list(args)
        if partition_name is not None:
            operands.append(b2j.partition_id_tensor())
        outs = b2j._bass_exec_p.bind(
            *operands,
            out_avals=tuple(out_avals),
            in_names=tuple(all_in),
            out_names=tuple(out_names),
            lowering_input_output_aliases=(),
            sim_require_finite=True,
            sim_require_nnan=True,
            nc=nc,
        )
        return tuple(outs)

    devices = jax.devices()[:NCORES]
    mesh = Mesh(_np.asarray(devices), ("core",))
    n_outs = len(out_names)
    donate = tuple(range(n_params, n_params + n_outs))
    sharded = jax.jit(
        shard_map(_body, mesh=mesh,
                  in_specs=(PartitionSpec("core"),) * (n_params + n_outs),
                  out_specs=(PartitionSpec("core"),) * n_outs,
                  check_rep=False),
        donate_argnums=donate, keep_unused=True)

    def run(in_maps):
        per_core = [[_np.asarray(m[name]) for name in in_names] for m in in_maps]
        concat_in = [
            _np.concatenate([per_core[c][i] for c in range(NCORES)], axis=0)
            for i in range(n_params)
        ]
        concat_zeros = [
            _np.zeros((NCORES * s[0], *s[1:]), dt) for (s, dt) in zero_shapes
        ]
        out_arrs = sharded(*concat_in, *concat_zeros)
        return [
            {name: _np.asarray(out_arrs[i]).reshape(NCORES, *out_avals[i].shape)[c]
             for i, name in enumerate(out_names)}
            for c in range(NCORES)
        ]

    nc._cached_exec = run
    return run


def _run_cached(in_maps, nc=None):
    if nc is None:
        nc = _get_program()
    return _cached_exec(nc)(in_maps)


def kernel(x, mixer, transfer, gain, _trace=False):
    in_maps = _host_inputs(x, mixer, transfer, gain)
    res = _run(in_maps, trace=_trace)
    out = np.empty((B, C, T), dtype=np.float32)
    for core in range(NCORES):
        b, h = divmod(core, 2)
        out[b, CH * h:CH * (h + 1), :] = \
            res.results[core]["out"].reshape(CH, T)
    if _trace:
        _CACHE["last_result"] = res
    return out



# revision 8
# speedup vs baseline: 1.1110x; 1.1110x over previous
"""Trainium2 Bass kernel for nn_Block_68633577390412.

Pipeline (reference): channel mix (64x64) -> frame into 256 half-overlapping
512-windows -> rfft -> per-(c,k) IIR over frames y_f = (s_f + y_{f-1})*t ->
irfft -> hann window -> overlap-add -> tanh(gain*.)

Sharding: 8 cores = 4 batches x 2 channel-halves, no cross-core traffic.

Per-core design (v2 — hop-block DFT + fp16 datapath):
  - Adjacent frames overlap by half a window (hop 256), so the rfft of frame
    f decomposes into two half-window transforms: S_f = A_f + (-1)^r A_{f+1}
    where A_h[r] = sum_{w<256} x[256h+w] e^{-i 2pi r w/512}. Each hop block's
    A is computed once (K=256 contraction, half the tensor-engine streams of
    the direct K=512 frame DFT); the +/- combine is a single fused
    scalar_tensor_tensor sweep on the vector engine.
  - Packed-real spectral rows are grouped by frequency parity
    (C0=Re-even, C1=Re-odd, C2=[Re_256|Im-even], C3=Im-odd) so (-1)^r is a
    constant sign per 128-row chunk and the combine needs no row masks.
  - The whole spectral datapath runs in fp16 (inputs, DFT bases, scan state
    s/y, transfer t, output): fp16 keeps ~1e-3 relative accuracy (tolerance
    is 2e-2) and unlocks the DVE 2x/4x wide modes plus half-size DMA.
  - The frame recurrence is the DVE TensorTensorScan instruction with fp32
    internal state, reading fp16 s tiles from SBUF.
  - irfft + hann + overlap-add stay fused as two accumulating matmul streams
    per output tile against precomputed E1/E2 = (irfft-basis * hann * gain)
    halves; tanh runs on the scalar engine over both channels at once and
    the output DMA stores fp16 with a (frame, channel, w) interleaved view.
  - All DMA goes through HWDGE (SP-engine issue, ~0.6us shared-device
    overhead) instead of Pool SWDGE (~1us Pool-engine hold each), freeing
    the Pool engine to zero scan-boundary columns.
  - Mix runs as a single fp16 stream (K=64) into PSUM, copied to the
    time-on-partitions fp16 signal buffer by the scalar engine; spectral
    work starts once the first ~17/32 of the signal is mixed and the
    remaining mix groups fill tensor-engine gaps between spectral pairs.
  - TRN2 instructions carry a single sync-wait slot; a post-pass hoists
    excess Tile-assigned waits onto standalone event-semaphore waits.
"""
import numpy as np
import ml_dtypes
from contextlib import ExitStack

import concourse.bass as bass
import concourse.tile as tile
from concourse import mybir
from concourse.bass_utils import run_bass_kernel_spmd

F32 = mybir.dt.float32
F16 = mybir.dt.float16

B, C, T = 4, 64, 65536
WS, STEP, K, NF = 512, 256, 257, 256
NCORES = 8
CH = 32            # channels per core
NCHUNK = 512       # 128-sample time chunks in T
XG = 32            # mix groups (2048 samples each)
NPAIR = CH // 2    # channel pairs per core

SIGMA = [1.0, -1.0, 1.0, -1.0]   # (-1)^r per packed row chunk C0..C3


def _packed_rows():
    """Packed-real row order: (type, r) per packed row, grouped so that
    (-1)^r is constant per 128-row chunk."""
    rows = []
    rows += [("re", r) for r in range(0, 256, 2)]          # C0: Re even
    rows += [("re", r) for r in range(1, 256, 2)]          # C1: Re odd
    rows += [("re", 256)] + [("im", r) for r in range(2, 256, 2)]  # C2
    rows += [("im", r) for r in range(1, 256, 2)]          # C3: Im odd
    assert len(rows) == 512
    return rows


def _build_matrices():
    rows = _packed_rows()
    w = np.arange(WS, dtype=np.float64)
    wh = w[:STEP]
    # D~ [256, 512]: half-window DFT basis, packed/parity-ordered columns.
    D = np.empty((STEP, 512), dtype=np.float64)
    for j, (ty, r) in enumerate(rows):
        if ty == "re":
            D[:, j] = np.cos(2 * np.pi * r * wh / WS)
        else:
            D[:, j] = -np.sin(2 * np.pi * r * wh / WS)
    # G [512, 512]: packed irfft basis rows in the same order.
    G = np.empty((512, WS), dtype=np.float64)
    for j, (ty, r) in enumerate(rows):
        if ty == "re":
            coef = 1.0 / WS if r in (0, 256) else 2.0 / WS
            G[j, :] = coef * np.cos(2 * np.pi * r * w / WS)
        else:
            G[j, :] = -(2.0 / WS) * np.sin(2 * np.pi * r * w / WS)
    hann = 0.5 * (1.0 - np.cos(2.0 * np.pi * w / WS))
    E1 = G[:, :STEP] * hann[None, :STEP]
    E2 = G[:, STEP:] * hann[None, STEP:]
    return D, E1, E2


def _split_excess_waits(nc):
    """TRN2 instructions have a single sync-wait slot. Tile's semaphore
    assignment can attach several waits to one instruction; hoist the extras
    onto standalone event-semaphore wait instructions on the same engine."""
    ctr = 0
    for fn in nc.m.functions:
        for bb in fn.blocks:
            out = []
            changed = False
            for inst in bb.instructions:
                si = inst.sync_info
                ow = list(si.on_wait) if si and si.on_wait else []
                if len(ow) > 1:
                    for wv in ow[:-1]:
                        ev = mybir.InstEventSemaphore(
                            name=f"WSPLIT-{ctr}", ins=[], outs=[])
                        ctr += 1
                        ev.engine = inst.engine
                        evsi = ev.sync_info
                        if evsi is None:
                            ev.sync_info = mybir.SyncInfo(on_wait=[wv],
                                                          on_update=[])
                        else:
                            evsi.on_wait = [wv]
                        out.append(ev)
                    si.on_wait = [ow[-1]]
                    changed = True
                out.append(inst)
            if changed:
                bb.instructions = out
    return ctr


def _build_program(split=True):
    nc = bass.Bass("TRN2", target_bir_lowering=False, debug=False,
                   num_devices=NCORES)
    xin_t = nc.dram_tensor("xin", [C, T], F16, kind="ExternalInput")
    msb_t = nc.dram_tensor("msb", [C, CH], F16, kind="ExternalInput")
    dmat_t = nc.dram_tensor("dmat", [STEP, 512], F16, kind="ExternalInput")
    emat_t = nc.dram_tensor("emat", [512, 512], F16, kind="ExternalInput")
    tpk_t = nc.dram_tensor("tpk", [512, CH], F16, kind="ExternalInput")
    out_t = nc.dram_tensor("out", [CH, 256, 256], F16, kind="ExternalOutput")

    xin = xin_t.ap()
    msb, dmat, emat, tpk = msb_t.ap(), dmat_t.ap(), emat_t.ap(), tpk_t.ap()

    with tile.TileContext(nc) as tc, ExitStack() as ctx:
        cpool = ctx.enter_context(tc.tile_pool(name="const", bufs=1))
        m_sb = cpool.tile([C, CH], F16, tag="m_sb")
        nc.sync.dma_start(m_sb[:], msb[:])
        dm_sb = cpool.tile([128, 1024], F16, tag="dm_sb")
        em_sb = cpool.tile([128, 2048], F16, tag="em_sb")
        tp_sb = cpool.tile([128, 4 * CH], F16, tag="tp_sb")
        for i in range(2):
            nc.sync.dma_start(dm_sb[:, 512 * i:512 * (i + 1)],
                              dmat[128 * i:128 * (i + 1), :])
        for i in range(4):
            nc.sync.dma_start(em_sb[:, 512 * i:512 * (i + 1)],
                              emat[128 * i:128 * (i + 1), :])
            nc.sync.dma_start(tp_sb[:, CH * i:CH * (i + 1)],
                              tpk[128 * i:128 * (i + 1), :])

        # x'' : mixed signal, time-on-partitions. col = 32*chunk + d_local.
        xsq = cpool.tile([128, CH * (NCHUNK + 2)], F16, tag="xsq")
        nc.vector.memset(xsq[:, CH * NCHUNK:], 0.0)

        xpool = ctx.enter_context(tc.tile_pool(name="xin", bufs=4))
        mixpool = ctx.enter_context(tc.tile_pool(name="mixps", bufs=2,
                                                 space="PSUM"))
        spool = ctx.enter_context(tc.tile_pool(name="aps", bufs=4,
                                               space="PSUM"))
        opool = ctx.enter_context(tc.tile_pool(name="ops", bufs=2,
                                               space="PSUM"))
        apool = ctx.enter_context(tc.tile_pool(name="asb", bufs=8))
        sbpool = ctx.enter_context(tc.tile_pool(name="ssb", bufs=8))
        ypool = ctx.enter_context(tc.tile_pool(name="y", bufs=16))
        outpool = ctx.enter_context(tc.tile_pool(name="outs", bufs=4))
        # per-(pair,c01,ri) scan carry between frame halves
        carry = cpool.tile([128, 8 * NPAIR], F16, tag="carry")

        xtiles = [None] * (XG // 2)

        def load_group(gt):
            xt = xpool.tile([C, 4096], F16, tag="xt")
            nc.sync.dma_start(xt[:], xin[:, 4096 * gt:4096 * (gt + 1)])
            xtiles[gt] = xt

        def mix_group(g):
            xt = xtiles[g // 2]
            base = 2048 * (g % 2)
            mps = mixpool.tile([128, 512], F32, tag="mps")
            for m in range(16):
                lhsT = xt[:, base + 128 * m:base + 128 * (m + 1)]
                nc.tensor.matmul(mps[:, CH * m:CH * (m + 1)], lhsT, m_sb[:],
                                 start=True, stop=True)
            nc.scalar.copy(xsq[:, 512 * g:512 * (g + 1)], mps[:])

        def spectral_front(p, half, act_copies=2):
            b0 = 128 * half            # first hop block of this half
            # A[r, (c01, blk)] for 129 blocks; one matmul per (ri, wi).
            s_tiles = []
            for ri in range(4):
                aps = spool.tile([128, 258], F32, tag="aps")
                for wi in range(2):
                    lhsT = dm_sb[:, 512 * wi + 128 * ri:
                                 512 * wi + 128 * (ri + 1)]
                    v = xsq[:, CH * (2 * b0 + wi) + 2 * p:
                            CH * (2 * b0 + wi) + 2 * p + 2]
                    rhs = bass.AP(v.tensor, v.offset,
                                  [list(q) for q in v.ap] + [[2 * CH, 129]])
                    nc.tensor.matmul(aps[:], lhsT, rhs,
                                     start=(wi == 0), stop=(wi == 1))
                # PSUM exit (only one PSUM src allowed per vector op): copy A
                # to fp16 SBUF, split between Act and DVE to balance engines.
                a_sb = apool.tile([128, 258], F16, tag="a")
                if ri < act_copies:
                    nc.scalar.copy(a_sb[:], aps[:])
                else:
                    nc.vector.tensor_copy(a_sb[:], aps[:])
                # s[:, c01*128+f] = A[:, c01*129+f] + sigma * A[:, c01*129+f+1]
                s_sb = sbpool.tile([128, 256], F16, tag="s")
                in0 = bass.AP(a_sb.tensor, a_sb.offset + 1,
                              [list(a_sb.ap[0]), [129, 2], [1, 128]])
                in1 = bass.AP(a_sb.tensor, a_sb.offset,
                              [list(a_sb.ap[0]), [129, 2], [1, 128]])
                outv = bass.AP(s_sb.tensor, s_sb.offset,
                               [list(s_sb.ap[0]), [128, 2], [1, 128]])
                nc.vector.scalar_tensor_tensor(
                    outv, in0, SIGMA[ri], in1,
                    mybir.AluOpType.mult, mybir.AluOpType.add)
                s_tiles.append(s_sb)

            y_tiles = {}
            for c01 in range(2):
                for ri in range(4):
                    yt = ypool.tile([128, 129], F16, tag="y")
                    cidx = p * 8 + c01 * 4 + ri
                    t_col = tp_sb[:, CH * ri + 2 * p + c01:
                                  CH * ri + 2 * p + c01 + 1]
                    if half == 0:
                        nc.gpsimd.memset(yt[:, 0:1], 0.0)
                        initial = 0.0
                    else:
                        nc.vector.tensor_copy(yt[:, 0:1],
                                              carry[:, cidx:cidx + 1])
                        initial = carry[:, cidx:cidx + 1]
                    nc.vector.tensor_tensor_scan(
                        yt[:, 1:129],
                        s_tiles[ri][:, 128 * c01:128 * (c01 + 1)],
                        t_col.broadcast_to((128, 128)),
                        initial, mybir.AluOpType.add, mybir.AluOpType.mult)
                    if half == 0:
                        nc.vector.tensor_copy(carry[:, cidx:cidx + 1],
                                              yt[:, 128:129])
                    y_tiles[(c01, ri)] = yt
            return y_tiles

        def spectral_back(p, half, y_tiles):
            # iDFT + hann + OLA; both channels in one PSUM tile.
            ops = opool.tile([128, 512], F32, tag="ops")
            for c01 in range(2):
                sl = ops[:, 256 * c01:256 * (c01 + 1)]
                for ri in range(4):
                    yt = y_tiles[(c01, ri)]
                    nc.tensor.matmul(sl, yt[:, 1:129],
                                     em_sb[:, 512 * ri:512 * ri + 256],
                                     start=(ri == 0), stop=False)
                    nc.tensor.matmul(sl, yt[:, 0:128],
                                     em_sb[:, 512 * ri + 256:512 * (ri + 1)],
                                     start=False, stop=(ri == 3))
            ot = outpool.tile([128, 512], F16, tag="ot")
            nc.scalar.activation(ot[:], ops[:],
                                 mybir.ActivationFunctionType.Tanh)
            # out[2p+c01, 128*half+f, :] <- ot[f, 256*c01:...]; dims (f,c01,w)
            oa = out_t.ap()
            dst = bass.AP(oa.tensor, (2 * p) * 65536 + (128 * half) * 256,
                          [[256, 128], [65536, 2], [1, 256]])
            nc.sync.dma_start(dst, ot[:])

        GSPLIT = 17          # mix groups needed before frame-half 0
        for gt in range(2):
            load_group(gt)
        for g in range(GSPLIT):
            if g % 2 == 0 and g // 2 + 2 < XG // 2:
                load_group(g // 2 + 2)
            mix_group(g)
        g_next = GSPLIT
        # half 0, software-pipelined by one pair; mix groups fill PE gaps
        pend = None
        for p in range(NPAIR):
            y0 = spectral_front(p, 0)
            if pend is not None:
                spectral_back(pend[0], 0, pend[1])
            pend = (p, y0)
            if g_next < XG:
                if g_next % 2 == 0 and g_next // 2 + 2 < XG // 2:
                    load_group(g_next // 2 + 2)
                mix_group(g_next)
                g_next += 1
        while g_next < XG:
            mix_group(g_next)
            g_next += 1
        spectral_back(pend[0], 0, pend[1])
        # half 1, software-pipelined by one pair
        pend = None
        for p in range(NPAIR):
            y1 = spectral_front(p, 1, act_copies=3)
            if pend is not None:
                spectral_back(pend[0], 1, pend[1])
            pend = (p, y1)
        spectral_back(pend[0], 1, pend[1])
    if split:
        _split_excess_waits(nc)
    return nc


_CACHE = {}


def _get_program():
    if "nc" not in _CACHE:
        _CACHE["nc"] = _build_program()
    return _CACHE["nc"]


def _host_inputs(x, mixer, transfer, gain):
    D, E1, E2 = _build_matrices()
    g = float(np.asarray(gain).reshape(-1)[0])
    dmat = np.ascontiguousarray(D, dtype=np.float16)
    emat = np.ascontiguousarray(
        np.concatenate([E1 * g, E2 * g], axis=1), dtype=np.float16)

    x = np.asarray(x, dtype=np.float32)
    mixer = np.asarray(mixer, dtype=np.float32)
    transfer = np.asarray(transfer, dtype=np.float32)

    rows = _packed_rows()
    r_of_row = np.array([r for (_, r) in rows], dtype=np.int64)

    in_maps = []
    for core in range(NCORES):
        b, h = divmod(core, 2)
        tr = transfer[CH * h:CH * (h + 1)]           # [32, 257]
        tpk = np.ascontiguousarray(tr[:, r_of_row].T,
                                   dtype=np.float16)  # [512, 32]
        in_maps.append({
            "xin": np.ascontiguousarray(x[b], dtype=np.float16),
            "msb": np.ascontiguousarray(mixer[:, CH * h:CH * (h + 1)],
                                        dtype=np.float16),
            "dmat": dmat,
            "emat": emat,
            "tpk": tpk,
        })
    return in_maps


def _run(in_maps, trace=False):
    nc = _get_program()
    return run_bass_kernel_spmd(nc, in_maps, list(range(NCORES)), trace=trace)


def kernel(x, mixer, transfer, gain, _trace=False):
    in_maps = _host_inputs(x, mixer, transfer, gain)
    res = _run(in_maps, trace=_trace)
    out = np.empty((B, C, T), dtype=np.float32)
    for core in range(NCORES):
        b, h = divmod(core, 2)
        out[b, CH * h:CH * (h + 1), :] = \
            res.results[core]["out"].astype(np.float32).reshape(CH, T)
    if _trace:
        _CACHE["last_result"] = res
    return out


# revision 12
# speedup vs baseline: 1.3568x; 1.2212x over previous
"""Trainium2 Bass kernel for nn_Block_68633577390412.

Pipeline (reference): channel mix (64x64) -> frame into 256 half-overlapping
512-windows -> rfft -> per-(c,k) IIR over frames y_f = (s_f + y_{f-1})*t ->
irfft -> hann window -> overlap-add -> tanh(gain*.)

Sharding: 8 cores = 4 batches x 2 channel-halves, no cross-core traffic.

Per-core design (v2 — hop-block DFT + fp16 datapath):
  - Adjacent frames overlap by half a window (hop 256), so the rfft of frame
    f decomposes into two half-window transforms: S_f = A_f + (-1)^r A_{f+1}
    where A_h[r] = sum_{w<256} x[256h+w] e^{-i 2pi r w/512}. Each hop block's
    A is computed once (K=256 contraction, half the tensor-engine streams of
    the direct K=512 frame DFT); the +/- combine is a single fused
    scalar_tensor_tensor sweep on the vector engine.
  - Packed-real spectral rows are grouped by frequency parity
    (C0=Re-even, C1=Re-odd, C2=[Re_256|Im-even], C3=Im-odd) so (-1)^r is a
    constant sign per 128-row chunk and the combine needs no row masks.
  - The whole spectral datapath runs in fp16 (inputs, DFT bases, scan state
    s/y, transfer t, output): fp16 keeps ~1e-3 relative accuracy (tolerance
    is 2e-2) and unlocks the DVE 2x/4x wide modes plus half-size DMA.
  - The frame recurrence is the DVE TensorTensorScan instruction with fp32
    internal state, reading fp16 s tiles from SBUF.
  - irfft + hann + overlap-add stay fused as two accumulating matmul streams
    per output tile against precomputed E1/E2 = (irfft-basis * hann * gain)
    halves; tanh runs on the scalar engine over both channels at once and
    the output DMA stores fp16 with a (frame, channel, w) interleaved view.
  - All DMA goes through HWDGE (SP-engine issue, ~0.6us shared-device
    overhead) instead of Pool SWDGE (~1us Pool-engine hold each), freeing
    the Pool engine to zero scan-boundary columns.
  - Mix runs as a single fp16 stream (K=64) into PSUM, copied to the
    time-on-partitions fp16 signal buffer by the scalar engine; spectral
    work starts once the first ~17/32 of the signal is mixed and the
    remaining mix groups fill tensor-engine gaps between spectral pairs.
  - TRN2 instructions carry a single sync-wait slot; a post-pass hoists
    excess Tile-assigned waits onto standalone event-semaphore waits.
"""
import numpy as np
import ml_dtypes
from contextlib import ExitStack

import concourse.bass as bass
import concourse.tile as tile
from concourse import mybir
from concourse.bass_utils import run_bass_kernel_spmd

F32 = mybir.dt.float32
F16 = mybir.dt.float16

B, C, T = 4, 64, 65536
WS, STEP, K, NF = 512, 256, 257, 256
NCORES = 8
CH = 32            # channels per core
NCHUNK = 512       # 128-sample time chunks in T
XG = 32            # mix groups (2048 samples each)
NPAIR = CH // 2    # channel pairs per core

SIGMA = [1.0, -1.0, 1.0, -1.0]   # (-1)^r per packed row chunk C0..C3


def _packed_rows():
    """Packed-real row order: (type, r) per packed row, grouped so that
    (-1)^r is constant per 128-row chunk."""
    rows = []
    rows += [("re", r) for r in range(0, 256, 2)]          # C0: Re even
    rows += [("re", r) for r in range(1, 256, 2)]          # C1: Re odd
    rows += [("re", 256)] + [("im", r) for r in range(2, 256, 2)]  # C2
    rows += [("im", r) for r in range(1, 256, 2)]          # C3: Im odd
    assert len(rows) == 512
    return rows


def _build_matrices():
    rows = _packed_rows()
    w = np.arange(WS, dtype=np.float64)
    wh = w[:STEP]
    # D~ [256, 512]: half-window DFT basis, packed/parity-ordered columns.
    D = np.empty((STEP, 512), dtype=np.float64)
    for j, (ty, r) in enumerate(rows):
        if ty == "re":
            D[:, j] = np.cos(2 * np.pi * r * wh / WS)
        else:
            D[:, j] = -np.sin(2 * np.pi * r * wh / WS)
    # G [512, 512]: packed irfft basis rows in the same order.
    G = np.empty((512, WS), dtype=np.float64)
    for j, (ty, r) in enumerate(rows):
        if ty == "re":
            coef = 1.0 / WS if r in (0, 256) else 2.0 / WS
            G[j, :] = coef * np.cos(2 * np.pi * r * w / WS)
        else:
            G[j, :] = -(2.0 / WS) * np.sin(2 * np.pi * r * w / WS)
    hann = 0.5 * (1.0 - np.cos(2.0 * np.pi * w / WS))
    E1 = G[:, :STEP] * hann[None, :STEP]
    E2 = G[:, STEP:] * hann[None, STEP:]
    return D, E1, E2


def _split_excess_waits(nc):
    """TRN2 instructions have a single sync-wait slot. Tile's semaphore
    assignment can attach several waits to one instruction; hoist the extras
    onto standalone event-semaphore wait instructions on the same engine."""
    ctr = 0
    for fn in nc.m.functions:
        for bb in fn.blocks:
            out = []
            changed = False
            for inst in bb.instructions:
                si = inst.sync_info
                ow = list(si.on_wait) if si and si.on_wait else []
                if len(ow) > 1:
                    for wv in ow[:-1]:
                        ev = mybir.InstEventSemaphore(
                            name=f"WSPLIT-{ctr}", ins=[], outs=[])
                        ctr += 1
                        ev.engine = inst.engine
                        evsi = ev.sync_info
                        if evsi is None:
                            ev.sync_info = mybir.SyncInfo(on_wait=[wv],
                                                          on_update=[])
                        else:
                            evsi.on_wait = [wv]
                        out.append(ev)
                    si.on_wait = [ow[-1]]
                    changed = True
                out.append(inst)
            if changed:
                bb.instructions = out
    return ctr


def _build_program(split=True):
    nc = bass.Bass("TRN2", target_bir_lowering=False, debug=False,
                   num_devices=NCORES)
    xin_t = nc.dram_tensor("xin", [C, T], F16, kind="ExternalInput")
    msb_t = nc.dram_tensor("msb", [C, CH], F16, kind="ExternalInput")
    dmat_t = nc.dram_tensor("dmat", [STEP, 512], F16, kind="ExternalInput")
    emat_t = nc.dram_tensor("emat", [512, 512], F16, kind="ExternalInput")
    tpk_t = nc.dram_tensor("tpk", [512, CH], F16, kind="ExternalInput")
    out_t = nc.dram_tensor("out", [CH, 256, 256], F16, kind="ExternalOutput")

    xin = xin_t.ap()
    msb, dmat, emat, tpk = msb_t.ap(), dmat_t.ap(), emat_t.ap(), tpk_t.ap()

    with tile.TileContext(nc) as tc, ExitStack() as ctx:
        cpool = ctx.enter_context(tc.tile_pool(name="const", bufs=1))
        m_sb = cpool.tile([C, CH], F16, tag="m_sb")
        dm_sb = cpool.tile([128, 1024], F16, tag="dm_sb")
        em_sb = cpool.tile([128, 2048], F16, tag="em_sb")
        tp_sb = cpool.tile([128, 4 * CH], F16, tag="tp_sb")

        # x'' : mixed signal, time-on-partitions. col = 32*chunk + d_local.
        xsq = cpool.tile([128, CH * (NCHUNK + 2)], F16, tag="xsq")
        nc.vector.memset(xsq[:, CH * NCHUNK:], 0.0)

        xpool = ctx.enter_context(tc.tile_pool(name="xin", bufs=4))

        def load_consts():
            # issued after the first x tiles: nothing here is needed until
            # the first DFT ~12us in, and HWDGE issue order gates the input.
            for i in range(2):
                nc.sync.dma_start(dm_sb[:, 512 * i:512 * (i + 1)],
                                  dmat[128 * i:128 * (i + 1), :])
            for i in range(4):
                nc.sync.dma_start(tp_sb[:, CH * i:CH * (i + 1)],
                                  tpk[128 * i:128 * (i + 1), :])
            for i in range(4):
                nc.sync.dma_start(em_sb[:, 512 * i:512 * (i + 1)],
                                  emat[128 * i:128 * (i + 1), :])
        mixpool = ctx.enter_context(tc.tile_pool(name="mixps", bufs=2,
                                                 space="PSUM"))
        spool = ctx.enter_context(tc.tile_pool(name="aps", bufs=4,
                                               space="PSUM"))
        opool = ctx.enter_context(tc.tile_pool(name="ops", bufs=2,
                                               space="PSUM"))
        apool = ctx.enter_context(tc.tile_pool(name="asb", bufs=8))
        sbpool = ctx.enter_context(tc.tile_pool(name="ssb", bufs=8))
        ypool = ctx.enter_context(tc.tile_pool(name="y", bufs=16))
        outpool = ctx.enter_context(tc.tile_pool(name="outs", bufs=4))
        # per-(pair,c01,ri) scan carry between frame halves
        carry = cpool.tile([128, 8 * NPAIR], F16, tag="carry")

        xtiles = [None] * (XG // 2)

        def load_group(gt):
            xt = xpool.tile([C, 4096], F16, tag="xt")
            nc.sync.dma_start(xt[:], xin[:, 4096 * gt:4096 * (gt + 1)])
            xtiles[gt] = xt

        def mix_group(g):
            xt = xtiles[g // 2]
            base = 2048 * (g % 2)
            mps = mixpool.tile([128, 512], F32, tag="mps")
            for m in range(16):
                lhsT = xt[:, base + 128 * m:base + 128 * (m + 1)]
                nc.tensor.matmul(mps[:, CH * m:CH * (m + 1)], lhsT, m_sb[:],
                                 start=True, stop=True)
            nc.scalar.copy(xsq[:, 512 * g:512 * (g + 1)], mps[:])

        def spectral_front(p, half, act_copies=2):
            b0 = 128 * half            # first hop block of this half
            # A[r, (c01, blk)] for 129 blocks; one matmul per (ri, wi).
            s_tiles = []
            for ri in range(4):
                aps = spool.tile([128, 258], F32, tag="aps")
                for wi in range(2):
                    lhsT = dm_sb[:, 512 * wi + 128 * ri:
                                 512 * wi + 128 * (ri + 1)]
                    v = xsq[:, CH * (2 * b0 + wi) + 2 * p:
                            CH * (2 * b0 + wi) + 2 * p + 2]
                    rhs = bass.AP(v.tensor, v.offset,
                                  [list(q) for q in v.ap] + [[2 * CH, 129]])
                    nc.tensor.matmul(aps[:], lhsT, rhs,
                                     start=(wi == 0), stop=(wi == 1))
                # PSUM exit (only one PSUM src allowed per vector op): copy A
                # to fp16 SBUF, split between Act and DVE to balance engines.
                a_sb = apool.tile([128, 258], F16, tag="a")
                if ri < act_copies:
                    nc.scalar.copy(a_sb[:], aps[:])
                else:
                    nc.vector.tensor_copy(a_sb[:], aps[:])
                # s[:, c01*128+f] = A[:, c01*129+f] +/- A[:, c01*129+f+1];
                # plain TensorTensor gets the DVE 2x fp16 mode.
                s_sb = sbpool.tile([128, 256], F16, tag="s")
                in0 = bass.AP(a_sb.tensor, a_sb.offset,
                              [list(a_sb.ap[0]), [129, 2], [1, 128]])
                in1 = bass.AP(a_sb.tensor, a_sb.offset + 1,
                              [list(a_sb.ap[0]), [129, 2], [1, 128]])
                outv = bass.AP(s_sb.tensor, s_sb.offset,
                               [list(s_sb.ap[0]), [128, 2], [1, 128]])
                op = (mybir.AluOpType.add if SIGMA[ri] > 0
                      else mybir.AluOpType.subtract)
                # the scan ISA only exists on DVE; give Pool a share of the
                # combines instead (half 0 has the most DVE pressure)
                tt_eng = nc.gpsimd if (half == 0 and ri < 2) else nc.vector
                tt_eng.tensor_tensor(outv, in0, in1, op)
                s_tiles.append(s_sb)

            y_tiles = {}
            for c01 in range(2):
                for ri in range(4):
                    yt = ypool.tile([128, 129], F16, tag="y")
                    cidx = p * 8 + c01 * 4 + ri
                    t_col = tp_sb[:, CH * ri + 2 * p + c01:
                                  CH * ri + 2 * p + c01 + 1]
                    if half == 0:
                        nc.vector.memset(yt[:, 0:1], 0.0)
                        initial = 0.0
                    else:
                        nc.vector.tensor_copy(yt[:, 0:1],
                                              carry[:, cidx:cidx + 1])
                        initial = carry[:, cidx:cidx + 1]
                    nc.vector.tensor_tensor_scan(
                        yt[:, 1:129],
                        s_tiles[ri][:, 128 * c01:128 * (c01 + 1)],
                        t_col.broadcast_to((128, 128)),
                        initial, mybir.AluOpType.add, mybir.AluOpType.mult)
                    if half == 0:
                        nc.vector.tensor_copy(carry[:, cidx:cidx + 1],
                                              yt[:, 128:129])
                    y_tiles[(c01, ri)] = yt
            return y_tiles

        def spectral_back(p, half, y_tiles):
            # iDFT + hann + OLA; both channels in one PSUM tile.
            ops = opool.tile([128, 512], F32, tag="ops")
            for c01 in range(2):
                sl = ops[:, 256 * c01:256 * (c01 + 1)]
                for ri in range(4):
                    yt = y_tiles[(c01, ri)]
                    nc.tensor.matmul(sl, yt[:, 1:129],
                                     em_sb[:, 512 * ri:512 * ri + 256],
                                     start=(ri == 0), stop=False)
                    nc.tensor.matmul(sl, yt[:, 0:128],
                                     em_sb[:, 512 * ri + 256:512 * (ri + 1)],
                                     start=False, stop=(ri == 3))
            ot = outpool.tile([128, 512], F16, tag="ot")
            nc.scalar.activation(ot[:], ops[:],
                                 mybir.ActivationFunctionType.Tanh)
            # out[2p+c01, 128*half+f, :] <- ot[f, 256*c01:...]; dims (f,c01,w)
            oa = out_t.ap()
            dst = bass.AP(oa.tensor, (2 * p) * 65536 + (128 * half) * 256,
                          [[256, 128], [65536, 2], [1, 256]])
            nc.sync.dma_start(dst, ot[:])

        GSPLIT = 17          # mix groups needed before frame-half 0
        nc.sync.dma_start(m_sb[:], msb[:])
        for gt in range(2):
            load_group(gt)
        load_consts()
        for g in range(GSPLIT):
            if g % 2 == 0 and g // 2 + 2 < XG // 2:
                load_group(g // 2 + 2)
            mix_group(g)
        g_next = GSPLIT
        # half 0, software-pipelined by one pair; mix groups fill PE gaps
        pend = None
        for p in range(NPAIR):
            y0 = spectral_front(p, 0)
            if pend is not None:
                spectral_back(pend[0], 0, pend[1])
            pend = (p, y0)
            if g_next < XG:
                if g_next % 2 == 0 and g_next // 2 + 2 < XG // 2:
                    load_group(g_next // 2 + 2)
                mix_group(g_next)
                g_next += 1
        while g_next < XG:
            mix_group(g_next)
            g_next += 1
        spectral_back(pend[0], 0, pend[1])
        # half 1, software-pipelined by one pair
        pend = None
        for p in range(NPAIR):
            y1 = spectral_front(p, 1, act_copies=3)
            if pend is not None:
                spectral_back(pend[0], 1, pend[1])
            pend = (p, y1)
        spectral_back(pend[0], 1, pend[1])
    if split:
        _split_excess_waits(nc)
    return nc


_CACHE = {}


def _get_program():
    if "nc" not in _CACHE:
        _CACHE["nc"] = _build_program()
    return _CACHE["nc"]


def _host_inputs(x, mixer, transfer, gain):
    D, E1, E2 = _build_matrices()
    g = float(np.asarray(gain).reshape(-1)[0])
    dmat = np.ascontiguousarray(D, dtype=np.float16)
    emat = np.ascontiguousarray(
        np.concatenate([E1 * g, E2 * g], axis=1), dtype=np.float16)

    x = np.asarray(x, dtype=np.float32)
    mixer = np.asarray(mixer, dtype=np.float32)
    transfer = np.asarray(transfer, dtype=np.float32)

    rows = _packed_rows()
    r_of_row = np.array([r for (_, r) in rows], dtype=np.int64)

    in_maps = []
    for core in range(NCORES):
        b, h = divmod(core, 2)
        tr = transfer[CH * h:CH * (h + 1)]           # [32, 257]
        tpk = np.ascontiguousarray(tr[:, r_of_row].T,
                                   dtype=np.float16)  # [512, 32]
        in_maps.append({
            "xin": np.ascontiguousarray(x[b], dtype=np.float16),
            "msb": np.ascontiguousarray(mixer[:, CH * h:CH * (h + 1)],
                                        dtype=np.float16),
            "dmat": dmat,
            "emat": emat,
            "tpk": tpk,
        })
    return in_maps


def _run(in_maps, trace=False):
    nc = _get_program()
    return run_bass_kernel_spmd(nc, in_maps, list(range(NCORES)), trace=trace)


def kernel(x, mixer, transfer, gain, _trace=False):
    in_maps = _host_inputs(x, mixer, transfer, gain)
    res = _run(in_maps, trace=_trace)
    out = np.empty((B, C, T), dtype=np.float32)
    for core in range(NCORES):
        b, h = divmod(core, 2)
        out[b, CH * h:CH * (h + 1), :] = \
            res.results[core]["out"].astype(np.float32).reshape(CH, T)
    if _trace:
        _CACHE["last_result"] = res
    return out
